# revision 43
# baseline (speedup 1.0000x reference)
"""Trainium2 Bass kernel for GQA attention (dense_transformer).

Full module: x[1,2048,4096] -> causal GQA attention (32 q heads, 8 kv heads,
head_dim 128, RoPE) -> out[1,2048,4096].

Sharding: tensor-parallel by heads across 8 NeuronCores. Core c owns q heads
4c..4c+3 and kv head c; wq/wk/wv column-sharded, wo row-sharded; x replicated.
The trailing all-reduce over wo partial sums is done host-side (outputs are
gathered to host anyway).

v2 design notes (vs the 3-phase v1 at ~766us):
  - All matmul operands are bf16 (PE runs at the same 1 cycle/row rate as
    fp32r, but DMA bytes, SBUF footprint and DVE op cost halve). PSUM
    accumulation stays fp32; rel-err gate is 2e-2 and bf16 lands ~1e-3.
  - Single fused pass: for each 512-wide seq chunk, QKV projection (+RoPE,
    +V transpose) is immediately followed by causal attention for that chunk
    (k/v of chunks 0..j are ready), with the wo projection at the end. The
    PE never idles long enough to re-throttle (HAM) and phase boundaries
    cost nothing.
  - DMA: ~60 large descriptors (vs 709 small) — weights resident in SBUF,
    x streamed in 1MB quarter-chunks double-buffered by emission order,
    outputs staged in 4-tile groups. Input DMAs ride the sync HWDGE queue;
    small startup tensors + wo weights + output stores ride the scalar
    HWDGE queue so the two FIFOs don't serialize against each other.
  - QKV runs output-major (all 32 d-tiles of one output block back to back)
    so 3 PSUM banks suffice and RoPE consumption trails two banks behind.
  - Attention: scores matmul writes only the causally-valid column range of
    the diagonal tiles; exp is a single ACT op per tile; the softmax
    denominator accumulates via an all-ones PE matmul (broadcast to all
    partitions); 1/d uses the custom-DVE reciprocal_approx_fast (~5x faster
    than the iterative divide, ~18 correct bits).
  - Max-subtraction is skipped: scores are O(+-10), exp cannot overflow.
"""

import math
from contextlib import ExitStack

import numpy as np

import concourse.bass as bass
import concourse.mybir as mybir
import concourse.tile as tile
from concourse import bacc, bass_utils

F32 = mybir.dt.float32
BF16 = mybir.dt.bfloat16

# Full-scale config (hardcoded; kernel.py must be self-contained).
DIM = 4096
SEQ = 2048
N_HEADS = 32
N_KV_HEADS = 8
HEAD_DIM = 128
N_CORES = 8
HQ = N_HEADS // N_CORES            # q heads per core = 4
CH = 512                           # seq chunk
SCALE = 1.0 / math.sqrt(HEAD_DIM)


def build_module(S=SEQ, D=DIM, hq=HQ, ch=CH):
    """Build the SPMD Bass/Tile module for one core's shard."""
    HD = HEAD_DIM
    H2 = HD // 2
    M = hq * HD                     # local q output dim (512)
    R = ch // 128                   # sk-tiles per sq chunk (4)
    nJ = S // ch                    # sq chunks (4)
    nT = S // 128                   # sk tiles (16)
    nD = D // 128                   # contraction tiles (32)
    NQ = nD // 4                    # d-tiles per x quarter (8)

    nc = bacc.Bacc("TRN2", target_bir_lowering=False, debug=False)
    # xP/wP are host-side pre-permuted so every DMA reads one long
    # contiguous run per partition (full HBM bandwidth):
    #   xP[j*4+qt, p, dd*ch+f] = x[(j*ch+f), (qt*NQ+dd)*128+p]
    #   wP[o, p, d*128+f]      = wqkv[(o*128+f), d*128+p]
    xP = nc.dram_tensor("xP", [nJ * 4, 128, NQ * ch], BF16,
                        kind="ExternalInput").ap()
    wP = nc.dram_tensor("wP", [6, 128, nD * 128], BF16,
                        kind="ExternalInput").ap()
    woT = nc.dram_tensor("woT", [M, D], BF16, kind="ExternalInput").ap()
    constD = nc.dram_tensor("constD", [128, 256], BF16, kind="ExternalInput").ap()
    cosP = nc.dram_tensor("cosP", [HD, S], BF16, kind="ExternalInput").ap()
    sinP = nc.dram_tensor("sinP", [HD, S], BF16, kind="ExternalInput").ap()
    maskD = nc.dram_tensor("maskD", [128, R * ch], BF16, kind="ExternalInput").ap()
    outT = nc.dram_tensor("outT", [D, S], BF16, kind="ExternalOutput").ap()

    woT_r = woT.rearrange("(o p) d -> p o d", p=128)      # [128, 4, D]
    outT_r = outT.rearrange("(g p) s -> p g s", p=128)    # [128, 32, S]

    with tile.TileContext(nc) as tc, ExitStack() as ctx, \
            nc.allow_low_precision(reason="bf16 operands, fp32 accumulation"):
        Exp = mybir.ActivationFunctionType.Exp

        # ---- persistent SBUF ----
        pers = ctx.enter_context(tc.tile_pool(name="pers", bufs=1))
        wsb = pers.tile([128, 6, nD * 128], BF16, tag="wsb", name="wsb")
        wosb = pers.tile([128, hq, D], BF16, tag="wosb", name="wosb")
        kT = pers.tile([HD, S], BF16, tag="kT", name="kT")
        vv = pers.tile([128, nT * HD], BF16, tag="vv", name="vv")
        yT = [pers.tile([HD, S], BF16, tag=f"yT{h}", name=f"yT{h}")
              for h in range(hq)]
        cosb = pers.tile([HD, S], BF16, tag="cosb", name="cosb")
        sinb = pers.tile([HD, S], BF16, tag="sinb", name="sinb")
        maskb = pers.tile([128, R * ch], BF16, tag="maskb", name="maskb")
        ident = pers.tile([128, 128], BF16, tag="ident", name="ident")
        ones128 = pers.tile([128, 128], BF16, tag="ones128", name="ones128")

        # Startup: chunk-0 x quarters and the first weight blocks gate the
        # first matmul streams, so interleave them across both HWDGE queues
        # (sync + scalar run in parallel) in consumption order.
        # Quarters 0/1 are double-buffered so the next chunk's prefetch can
        # issue at chunk start; 2/3 single-buffered (prefetched after their
        # last use in the v stream) to stay inside SBUF.
        #
        # Startup is HBM-bound (10MB of weights+x gate chunk 0), so the
        # DMAs are interleaved across both HWDGE rings in the order the
        # matmul streams consume them: x eighths first (q0 reads all of x),
        # weight blocks just in time, small tensors last.
        xpool = ctx.enter_context(tc.tile_pool(name="xpool", bufs=1))
        xq_bufs = {0: 2, 1: 2, 2: 1, 3: 1}
        cur_xq = {}
        H8 = NQ // 2
        for qt in range(4):
            cur_xq[qt] = xpool.tile([128, NQ, ch], BF16, tag=f"xq{qt}",
                                    name=f"xq{qt}", bufs=xq_bufs[qt])
        # chunk 0 in eighths: the first matmuls start right after the
        # first 0.5MB lands instead of waiting for a full quarter
        for qt in range(4):
            nc.scalar.dma_start(cur_xq[qt][:, 0:H8, :],
                                xP[qt][:, 0:H8 * ch])
            nc.scalar.dma_start(cur_xq[qt][:, H8:NQ, :],
                                xP[qt][:, H8 * ch:NQ * ch])
        for o in range(6):
            nc.sync.dma_start(wsb[:, o, :], wP[o])
        nc.scalar.dma_start(cosb[:], cosP[:])
        nc.scalar.dma_start(sinb[:], sinP[:])
        nc.scalar.dma_start(maskb[:], maskD[:])
        nc.scalar.dma_start(ident[:], constD[:, 0:128])
        nc.scalar.dma_start(ones128[:], constD[:, 128:256])
        nc.scalar.dma_start(wosb[:], woT_r[:])

        qtpool = ctx.enter_context(tc.tile_pool(name="qtpool", bufs=2))
        rpool = ctx.enter_context(tc.tile_pool(name="rpool", bufs=2))
        vpool = ctx.enter_context(tc.tile_pool(name="vpool", bufs=2))
        apool = ctx.enter_context(tc.tile_pool(name="apool", bufs=8))
        npool = ctx.enter_context(tc.tile_pool(name="npool", bufs=2))

        # Attention PSUM pool is opened first (5 banks: scores 2, y 2, d 1);
        # the qkv pool (3 banks) nests inside and frees its banks to wo_ps,
        # which coexists with attn_ps so late attention overlaps wo.
        attn_ps = ctx.enter_context(
            tc.tile_pool(name="attn_ps", bufs=1, space="PSUM"))

        def rope(out, ps, j):
            """out[:, chunk] = RoPE(ps) with de-interleaved halves.

            The 64-partition swap pairs a PSUM operand with an SBUF operand
            (mixed-space ops may differ in base partition; SB+SB must not).
            """
            cj = cosb[:, j * ch:(j + 1) * ch]
            sj = sinb[:, j * ch:(j + 1) * ch]
            nc.vector.tensor_mul(out, ps[:], cj)
            tmp = rpool.tile([HD, ch], BF16, tag="ropetmp", name="ropetmp")
            nc.vector.tensor_mul(tmp[0:H2, :], ps[H2:HD, :], sj[0:H2, :])
            nc.vector.tensor_mul(tmp[H2:HD, :], ps[0:H2, :], sj[H2:HD, :])
            nc.vector.tensor_add(out, out, tmp[:])

        def qkv_chunk(j, qkv_ps, qTc):
            # Output-major: all 32 d-tile matmuls of one output block run
            # back to back; psum banks rotate a,b,c so RoPE trails 2 behind.
            use_xq = dict(cur_xq)  # this chunk's buffers
            # double-buffered quarters prefetch at chunk start
            if j + 1 < nJ:
                for qt in (0, 1):
                    nxt = xpool.tile([128, NQ, ch], BF16, tag=f"xq{qt}",
                                     name=f"xq{qt}", bufs=xq_bufs[qt])
                    nc.sync.dma_start(nxt[:], xP[(j + 1) * 4 + qt])
                    cur_xq[qt] = nxt
            tags = ["a", "b", "c", "a", "b", "c"]
            for oi in range(6):
                ps = qkv_ps.tile([128, ch], F32, tag=tags[oi], name=f"ps{oi}")
                for dd in range(nD):
                    xt = use_xq[dd // NQ]
                    nc.tensor.matmul(
                        ps[:], wsb[:, oi, dd * 128:(dd + 1) * 128],
                        xt[:, dd % NQ, :],
                        start=(dd == 0), stop=(dd == nD - 1))
                    # v is the last reader of the single-buffered quarters:
                    # prefetch the next chunk's right after their final use.
                    if (oi == 5 and dd % NQ == NQ - 1 and j + 1 < nJ
                            and dd // NQ in (2, 3)):
                        qt = dd // NQ
                        nxt = xpool.tile([128, NQ, ch], BF16, tag=f"xq{qt}",
                                         name=f"xq{qt}", bufs=xq_bufs[qt])
                        nc.sync.dma_start(nxt[:], xP[(j + 1) * 4 + qt])
                        cur_xq[qt] = nxt
                if oi < hq:
                    rope(qTc[oi][:], ps, j)
                elif oi == hq:
                    rope(kT[:, j * ch:(j + 1) * ch], ps, j)
                else:
                    # v: psum [hd, ch] -> sbuf bf16, then PE-transpose per
                    # 128 block into one psum bank (reuses tag "a"), then one
                    # copy into the persistent [sk, hd] v layout.
                    vt_s = vpool.tile([HD, ch], BF16, tag="vts", name="vts")
                    nc.vector.tensor_copy(vt_s[:], ps[:])
                    pvt = qkv_ps.tile([128, ch], BF16, tag="a", name="pvt")
                    for r in range(R):
                        nc.tensor.transpose(
                            pvt[:, r * 128:(r + 1) * 128],
                            vt_s[:, r * 128:(r + 1) * 128], ident[:])
                    nc.vector.tensor_copy(
                        vv[:, j * R * HD:(j + 1) * R * HD], pvt[:])

        def attn_chunk(j, qTc):
            # Transposed flash-style causal attention for the 4 local heads.
            # Scores land transposed (sk on partitions) so P@V needs no
            # transpose; softmax denominator accumulates on the PE via an
            # all-ones lhsT (broadcasts the column sum to every partition).
            nTj = (j + 1) * R
            for h in range(hq):
                y_ps = attn_ps.tile([HD, ch], F32, tag="yps", name="yps",
                                    bufs=2)
                ps_d = attn_ps.tile([128, ch], F32, tag="dps", name="dps")
                qsl = qTc[h]

                def score(t):
                    # Diagonal tiles only have valid scores at sq >= 128*r.
                    off = max(0, (t - j * R) * 128)
                    s_ps = attn_ps.tile([128, ch], F32, tag="sps",
                                        name="sps", bufs=2)
                    nc.tensor.matmul(
                        s_ps[:, off:ch], kT[:, t * 128:(t + 1) * 128],
                        qsl[:, off:ch], start=True, stop=True)
                    return s_ps, off

                pipe = [score(0)]
                for t in range(nTj):
                    s_ps, off = pipe[t]
                    if t + 1 < nTj:
                        pipe.append(score(t + 1))
                    et = apool.tile([128, ch], BF16, tag="et", name="et")
                    # scale folded into wq host-side; ACT does pure exp
                    nc.scalar.activation(et[:, off:ch], s_ps[:, off:ch], Exp)
                    r = t - j * R
                    if r >= 0:  # diagonal tile: apply causal mask
                        nc.vector.tensor_mul(
                            et[:, off:ch], et[:, off:ch],
                            maskb[:, r * ch + off:(r + 1) * ch])
                    st, sp = (t == 0), (t == nTj - 1)
                    nc.tensor.matmul(ps_d[:, off:ch], ones128[:],
                                     et[:, off:ch], start=st, stop=sp)
                    nc.tensor.matmul(y_ps[:, off:ch],
                                     vv[:, t * HD:(t + 1) * HD],
                                     et[:, off:ch], start=st, stop=sp)
                # Copy d out of PSUM promptly (frees the bank for the next
                # head) and take the fast approximate reciprocal in SBUF.
                d_sb = npool.tile([128, ch], F32, tag="dsb", name="dsb")
                nc.vector.tensor_copy(d_sb[:], ps_d[:])
                rec = npool.tile([128, ch], F32, tag="rec", name="rec")
                nc.vector.reciprocal_approx_fast(rec[:], d_sb[:])
                nc.vector.tensor_mul(
                    yT[h][:, j * ch:(j + 1) * ch], y_ps[:], rec[:])

        with tc.tile_pool(name="qkv_ps", bufs=1, space="PSUM") as qkv_ps:
            for j in range(nJ):
                qTc = [qtpool.tile([HD, ch], BF16, tag=f"qt{h}",
                                   name=f"qt{h}") for h in range(hq)]
                qkv_chunk(j, qkv_ps, qTc)
                attn_chunk(j, qTc)

        # ---- output projection (row-parallel wo partial sums) ----
        opool = ctx.enter_context(tc.tile_pool(name="opool", bufs=2))
        with tc.tile_pool(name="wo_ps", bufs=1, space="PSUM") as wo_ps:
            for j in range(nJ):
                for g in range(nD // 4):
                    last = (j == nJ - 1 and g == nD // 4 - 1)
                    og = opool.tile([128, 4, ch], BF16, tag="og", name="og")
                    for i in range(4):
                        dt = g * 4 + i
                        ps_o = wo_ps.tile([128, ch], F32, tag="pso",
                                          name="pso", bufs=3)
                        for o in range(hq):
                            nc.tensor.matmul(
                                ps_o[:], wosb[:, o, dt * 128:(dt + 1) * 128],
                                yT[o][:, j * ch:(j + 1) * ch],
                                start=(o == 0), stop=(o == hq - 1))
                        if last:
                            # final group: split each evacuation across both
                            # engines to shorten the kernel tail
                            h2c = ch // 2
                            nc.vector.tensor_copy(og[:, i, 0:h2c],
                                                  ps_o[:, 0:h2c])
                            nc.scalar.copy(og[:, i, h2c:ch], ps_o[:, h2c:ch])
                        elif dt % 2:
                            nc.scalar.copy(og[:, i, :], ps_o[:])
                        else:
                            nc.vector.tensor_copy(og[:, i, :], ps_o[:])
                    if last:
                        nc.scalar.dma_start(
                            outT_r[:, g * 4:g * 4 + 2, j * ch:(j + 1) * ch],
                            og[:, 0:2, :])
                        nc.scalar.dma_start(
                            outT_r[:, g * 4 + 2:g * 4 + 4,
                                   j * ch:(j + 1) * ch],
                            og[:, 2:4, :])
                    else:
                        nc.scalar.dma_start(
                            outT_r[:, g * 4:(g + 1) * 4,
                                   j * ch:(j + 1) * ch],
                            og[:])
    nc.compile()
    return nc


def _deinterleave_perm(hd):
    """Row permutation putting even indices first, odd second."""
    return np.concatenate([np.arange(0, hd, 2), np.arange(1, hd, 2)])


def host_prep(x, wq, wk, wv, wo, freqs_cos, freqs_sin,
              n_cores=N_CORES, hq=HQ, n_kv=N_KV_HEADS):
    """Build the per-core input maps (numpy, host-side)."""
    import ml_dtypes

    BF = ml_dtypes.bfloat16
    HD = HEAD_DIM
    D = x.shape[-1]
    S = x.shape[-2]
    M = hq * HD
    R = CH // 128
    x = np.asarray(x, np.float32).reshape(S, D)
    wq = np.asarray(wq, np.float32)
    wk = np.asarray(wk, np.float32)
    wv = np.asarray(wv, np.float32)
    wo = np.asarray(wo, np.float32)
    fc = np.asarray(freqs_cos, np.float32)
    fs = np.asarray(freqs_sin, np.float32)

    perm = _deinterleave_perm(HD)
    wq = wq * np.float32(SCALE)   # fold softmax scale into q projection
    # xP[j*4+qt, p, dd*CH+f] = x[j*CH+f, (qt*8+dd)*128+p] — one contiguous
    # 8KB run per (chunk-quarter, partition) so the DMA hits full HBM BW.
    nJ = S // CH
    xP = np.ascontiguousarray(
        x.T.reshape(4, 8, 128, nJ, CH).transpose(3, 0, 2, 1, 4)
        .reshape(nJ * 4, 128, 8 * CH)).astype(BF)
    cosP = np.ascontiguousarray(np.concatenate([fc.T, fc.T], 0)).astype(BF)
    sinP = np.ascontiguousarray(np.concatenate([-fs.T, fs.T], 0)).astype(BF)
    # mask[t, r*CH + s] = 1 if 128*r + t <= s else 0
    tt = np.arange(128)[:, None]
    ss = np.arange(CH)[None, :]
    maskD = np.concatenate(
        [(128 * r + tt <= ss).astype(np.float32) for r in range(R)], axis=1)
    maskD = np.ascontiguousarray(maskD).astype(BF)            # [128, R*CH]
    constD = np.concatenate(
        [np.eye(128, dtype=np.float32), np.ones((128, 128), np.float32)],
        axis=1).astype(BF)                                    # [128, 256]

    nD = D // 128
    in_maps = []
    for c in range(n_cores):
        wq_c = wq[c * M:(c + 1) * M, :].reshape(hq, HD, D)[:, perm, :]
        wq_c = wq_c.reshape(M, D)
        wk_c = wk[c * HD:(c + 1) * HD, :][perm, :]
        wv_c = wv[c * HD:(c + 1) * HD, :]
        wqkvT = np.concatenate([wq_c, wk_c, wv_c], axis=0).T  # [D, 768]
        # wP[o, p, d*128+f] = wqkvT[d*128+p, o*128+f]
        wP = np.ascontiguousarray(
            wqkvT.reshape(nD, 128, 6, 128).transpose(2, 1, 0, 3)
            .reshape(6, 128, nD * 128)).astype(BF)
        woT = np.ascontiguousarray(wo[:, c * M:(c + 1) * M].T).astype(BF)
        in_maps.append({
            "xP": xP, "wP": wP, "woT": woT, "constD": constD,
            "cosP": cosP, "sinP": sinP, "maskD": maskD,
        })
    return in_maps


_NC_CACHE = {}


def _get_module():
    if "nc" not in _NC_CACHE:
        _NC_CACHE["nc"] = build_module()
    return _NC_CACHE["nc"]


def run_on_cores(in_maps, trace=False):
    nc = _get_module()
    res = bass_utils.run_bass_kernel_spmd(
        nc, in_maps, core_ids=list(range(len(in_maps))), trace=trace)
    return res


def kernel(x, wq, wk, wv, wo, freqs_cos, freqs_sin):
    in_maps = host_prep(x, wq, wk, wv, wo, freqs_cos, freqs_sin)
    res = run_on_cores(in_maps)
    acc = None
    for r in res.results:
        o = np.asarray(r["outT"], dtype=np.float64)
        acc = o if acc is None else acc + o
    out = acc.T.astype(np.float32).reshape(1, SEQ, DIM)
    return out


# revision 44
# speedup vs baseline: 1.0050x; 1.0050x over previous
"""Trainium2 Bass kernel for GQA attention (dense_transformer).

Full module: x[1,2048,4096] -> causal GQA attention (32 q heads, 8 kv heads,
head_dim 128, RoPE) -> out[1,2048,4096].

Sharding: tensor-parallel by heads across 8 NeuronCores. Core c owns q heads
4c..4c+3 and kv head c; wq/wk/wv column-sharded, wo row-sharded; x replicated.
The trailing all-reduce over wo partial sums is done host-side (outputs are
gathered to host anyway).

v2 design notes (vs the 3-phase v1 at ~766us):
  - All matmul operands are bf16 (PE runs at the same 1 cycle/row rate as
    fp32r, but DMA bytes, SBUF footprint and DVE op cost halve). PSUM
    accumulation stays fp32; rel-err gate is 2e-2 and bf16 lands ~1e-3.
  - Single fused pass: for each 512-wide seq chunk, QKV projection (+RoPE,
    +V transpose) is immediately followed by causal attention for that chunk
    (k/v of chunks 0..j are ready), with the wo projection at the end. The
    PE never idles long enough to re-throttle (HAM) and phase boundaries
    cost nothing.
  - DMA: ~60 large descriptors (vs 709 small) — weights resident in SBUF,
    x streamed in 1MB quarter-chunks double-buffered by emission order,
    outputs staged in 4-tile groups. Input DMAs ride the sync HWDGE queue;
    small startup tensors + wo weights + output stores ride the scalar
    HWDGE queue so the two FIFOs don't serialize against each other.
  - QKV runs output-major (all 32 d-tiles of one output block back to back)
    so 3 PSUM banks suffice and RoPE consumption trails two banks behind.
  - Attention: scores matmul writes only the causally-valid column range of
    the diagonal tiles; exp is a single ACT op per tile; the softmax
    denominator accumulates via an all-ones PE matmul (broadcast to all
    partitions); 1/d uses the custom-DVE reciprocal_approx_fast (~5x faster
    than the iterative divide, ~18 correct bits).
  - Max-subtraction is skipped: scores are O(+-10), exp cannot overflow.
"""

import math
from contextlib import ExitStack

import numpy as np

import concourse.bass as bass
import concourse.mybir as mybir
import concourse.tile as tile
from concourse import bacc, bass_utils

F32 = mybir.dt.float32
BF16 = mybir.dt.bfloat16

# Full-scale config (hardcoded; kernel.py must be self-contained).
DIM = 4096
SEQ = 2048
N_HEADS = 32
N_KV_HEADS = 8
HEAD_DIM = 128
N_CORES = 8
HQ = N_HEADS // N_CORES            # q heads per core = 4
CH = 512                           # seq chunk
SCALE = 1.0 / math.sqrt(HEAD_DIM)


def build_module(S=SEQ, D=DIM, hq=HQ, ch=CH):
    """Build the SPMD Bass/Tile module for one core's shard."""
    HD = HEAD_DIM
    H2 = HD // 2
    M = hq * HD                     # local q output dim (512)
    R = ch // 128                   # sk-tiles per sq chunk (4)
    nJ = S // ch                    # sq chunks (4)
    nT = S // 128                   # sk tiles (16)
    nD = D // 128                   # contraction tiles (32)
    NQ = nD // 4                    # d-tiles per x quarter (8)

    nc = bacc.Bacc("TRN2", target_bir_lowering=False, debug=False)
    # xP/wP are host-side pre-permuted so every DMA reads one long
    # contiguous run per partition (full HBM bandwidth):
    #   xP[j*4+qt, p, dd*ch+f] = x[(j*ch+f), (qt*NQ+dd)*128+p]
    #   wP[o, p, d*128+f]      = wqkv[(o*128+f), d*128+p]
    xP = nc.dram_tensor("xP", [nJ * 4, 128, NQ * ch], BF16,
                        kind="ExternalInput").ap()
    wP = nc.dram_tensor("wP", [6, 128, nD * 128], BF16,
                        kind="ExternalInput").ap()
    woT = nc.dram_tensor("woT", [M, D], BF16, kind="ExternalInput").ap()
    constD = nc.dram_tensor("constD", [128, 256], BF16, kind="ExternalInput").ap()
    cosP = nc.dram_tensor("cosP", [HD, S], BF16, kind="ExternalInput").ap()
    sinP = nc.dram_tensor("sinP", [HD, S], BF16, kind="ExternalInput").ap()
    maskD = nc.dram_tensor("maskD", [128, R * ch], BF16, kind="ExternalInput").ap()
    outT = nc.dram_tensor("outT", [D, S], BF16, kind="ExternalOutput").ap()

    woT_r = woT.rearrange("(o p) d -> p o d", p=128)      # [128, 4, D]
    outT_r = outT.rearrange("(g p) s -> p g s", p=128)    # [128, 32, S]

    with tile.TileContext(nc) as tc, ExitStack() as ctx, \
            nc.allow_low_precision(reason="bf16 operands, fp32 accumulation"):
        Exp = mybir.ActivationFunctionType.Exp

        # ---- persistent SBUF ----
        pers = ctx.enter_context(tc.tile_pool(name="pers", bufs=1))
        wsb = pers.tile([128, 6, nD * 128], BF16, tag="wsb", name="wsb")
        wosb = pers.tile([128, hq, D], BF16, tag="wosb", name="wosb")
        kT = pers.tile([HD, S], BF16, tag="kT", name="kT")
        vv = pers.tile([128, nT * HD], BF16, tag="vv", name="vv")
        yT = [pers.tile([HD, S], BF16, tag=f"yT{h}", name=f"yT{h}")
              for h in range(hq)]
        cosb = pers.tile([HD, S], BF16, tag="cosb", name="cosb")
        sinb = pers.tile([HD, S], BF16, tag="sinb", name="sinb")
        maskb = pers.tile([128, R * ch], BF16, tag="maskb", name="maskb")
        ident = pers.tile([128, 128], BF16, tag="ident", name="ident")
        ones128 = pers.tile([128, 128], BF16, tag="ones128", name="ones128")

        # Startup: chunk-0 x quarters and the first weight blocks gate the
        # first matmul streams, so interleave them across both HWDGE queues
        # (sync + scalar run in parallel) in consumption order.
        # Quarters 0/1 are double-buffered so the next chunk's prefetch can
        # issue at chunk start; 2/3 single-buffered (prefetched after their
        # last use in the v stream) to stay inside SBUF.
        #
        # Startup is HBM-bound (10MB of weights+x gate chunk 0), so the
        # DMAs are interleaved across both HWDGE rings in the order the
        # matmul streams consume them: x eighths first (q0 reads all of x),
        # weight blocks just in time, small tensors last.
        xpool = ctx.enter_context(tc.tile_pool(name="xpool", bufs=1))
        xq_bufs = {0: 2, 1: 2, 2: 1, 3: 1}
        cur_xq = {}
        H8 = NQ // 2
        for qt in range(4):
            cur_xq[qt] = xpool.tile([128, NQ, ch], BF16, tag=f"xq{qt}",
                                    name=f"xq{qt}", bufs=xq_bufs[qt])
        # chunk 0 in eighths: the first matmuls start right after the
        # first 0.5MB lands instead of waiting for a full quarter
        for qt in range(4):
            nc.scalar.dma_start(cur_xq[qt][:, 0:H8, :],
                                xP[qt][:, 0:H8 * ch])
            nc.scalar.dma_start(cur_xq[qt][:, H8:NQ, :],
                                xP[qt][:, H8 * ch:NQ * ch])
        for o in range(6):
            nc.sync.dma_start(wsb[:, o, :], wP[o])
        nc.scalar.dma_start(cosb[:], cosP[:])
        nc.scalar.dma_start(sinb[:], sinP[:])
        nc.scalar.dma_start(maskb[:], maskD[:])
        nc.scalar.dma_start(ident[:], constD[:, 0:128])
        nc.scalar.dma_start(ones128[:], constD[:, 128:256])
        nc.scalar.dma_start(wosb[:], woT_r[:])

        qtpool = ctx.enter_context(tc.tile_pool(name="qtpool", bufs=2))
        rpool = ctx.enter_context(tc.tile_pool(name="rpool", bufs=2))
        vpool = ctx.enter_context(tc.tile_pool(name="vpool", bufs=2))
        apool = ctx.enter_context(tc.tile_pool(name="apool", bufs=6))
        npool = ctx.enter_context(tc.tile_pool(name="npool", bufs=2))

        # Attention PSUM pool is opened first (5 banks: scores 2, y 2, d 1);
        # the qkv pool (3 banks) nests inside and frees its banks to wo_ps,
        # which coexists with attn_ps so late attention overlaps wo.
        attn_ps = ctx.enter_context(
            tc.tile_pool(name="attn_ps", bufs=1, space="PSUM"))

        def rope(out, ps, j):
            """out[:, chunk] = RoPE(ps) with de-interleaved halves.

            The 64-partition swap pairs a PSUM operand with an SBUF operand
            (mixed-space ops may differ in base partition; SB+SB must not).
            """
            cj = cosb[:, j * ch:(j + 1) * ch]
            sj = sinb[:, j * ch:(j + 1) * ch]
            nc.vector.tensor_mul(out, ps[:], cj)
            tmp = rpool.tile([HD, ch], BF16, tag="ropetmp", name="ropetmp")
            nc.vector.tensor_mul(tmp[0:H2, :], ps[H2:HD, :], sj[0:H2, :])
            nc.vector.tensor_mul(tmp[H2:HD, :], ps[0:H2, :], sj[H2:HD, :])
            nc.vector.tensor_add(out, out, tmp[:])

        def qkv_chunk(j, qkv_ps, qTc):
            # Output-major: all 32 d-tile matmuls of one output block run
            # back to back; psum banks rotate a,b,c so RoPE trails 2 behind.
            use_xq = dict(cur_xq)  # this chunk's buffers
            # double-buffered quarters prefetch at chunk start
            if j + 1 < nJ:
                for qt in (0, 1):
                    nxt = xpool.tile([128, NQ, ch], BF16, tag=f"xq{qt}",
                                     name=f"xq{qt}", bufs=xq_bufs[qt])
                    nc.sync.dma_start(nxt[:], xP[(j + 1) * 4 + qt])
                    cur_xq[qt] = nxt
            tags = ["a", "b", "c", "a", "b", "c"]
            for oi in range(6):
                ps = qkv_ps.tile([128, ch], F32, tag=tags[oi], name=f"ps{oi}")
                for dd in range(nD):
                    xt = use_xq[dd // NQ]
                    nc.tensor.matmul(
                        ps[:], wsb[:, oi, dd * 128:(dd + 1) * 128],
                        xt[:, dd % NQ, :],
                        start=(dd == 0), stop=(dd == nD - 1))
                    # v is the last reader of the single-buffered quarters:
                    # prefetch the next chunk's right after their final use.
                    if (oi == 5 and dd % NQ == NQ - 1 and j + 1 < nJ
                            and dd // NQ in (2, 3)):
                        qt = dd // NQ
                        nxt = xpool.tile([128, NQ, ch], BF16, tag=f"xq{qt}",
                                         name=f"xq{qt}", bufs=xq_bufs[qt])
                        nc.sync.dma_start(nxt[:], xP[(j + 1) * 4 + qt])
                        cur_xq[qt] = nxt
                if oi < hq:
                    rope(qTc[oi][:], ps, j)
                elif oi == hq:
                    rope(kT[:, j * ch:(j + 1) * ch], ps, j)
                else:
                    # v: psum [hd, ch] -> sbuf bf16, then PE-transpose per
                    # 128 block into one psum bank (reuses tag "a"), then one
                    # copy into the persistent [sk, hd] v layout.
                    vt_s = vpool.tile([HD, ch], BF16, tag="vts", name="vts")
                    nc.vector.tensor_copy(vt_s[:], ps[:])
                    pvt = qkv_ps.tile([128, ch], BF16, tag="a", name="pvt")
                    for r in range(R):
                        nc.tensor.transpose(
                            pvt[:, r * 128:(r + 1) * 128],
                            vt_s[:, r * 128:(r + 1) * 128], ident[:])
                    nc.vector.tensor_copy(
                        vv[:, j * R * HD:(j + 1) * R * HD], pvt[:])

        def attn_chunk(j, qTc):
            # Transposed flash-style causal attention for the 4 local heads.
            # Scores land transposed (sk on partitions) so P@V needs no
            # transpose; softmax denominator accumulates on the PE via an
            # all-ones lhsT (broadcasts the column sum to every partition).
            nTj = (j + 1) * R
            for h in range(hq):
                y_ps = attn_ps.tile([HD, ch], F32, tag="yps", name="yps",
                                    bufs=2)
                ps_d = attn_ps.tile([128, ch], F32, tag="dps", name="dps")
                qsl = qTc[h]

                def score(t):
                    # Diagonal tiles only have valid scores at sq >= 128*r.
                    off = max(0, (t - j * R) * 128)
                    s_ps = attn_ps.tile([128, ch], F32, tag="sps",
                                        name="sps", bufs=2)
                    nc.tensor.matmul(
                        s_ps[:, off:ch], kT[:, t * 128:(t + 1) * 128],
                        qsl[:, off:ch], start=True, stop=True)
                    return s_ps, off

                pipe = [score(0)]
                for t in range(nTj):
                    s_ps, off = pipe[t]
                    if t + 1 < nTj:
                        pipe.append(score(t + 1))
                    et = apool.tile([128, ch], BF16, tag="et", name="et")
                    # scale folded into wq host-side; ACT does pure exp
                    nc.scalar.activation(et[:, off:ch], s_ps[:, off:ch], Exp)
                    r = t - j * R
                    if r >= 0:  # diagonal tile: apply causal mask
                        nc.vector.tensor_mul(
                            et[:, off:ch], et[:, off:ch],
                            maskb[:, r * ch + off:(r + 1) * ch])
                    st, sp = (t == 0), (t == nTj - 1)
                    nc.tensor.matmul(ps_d[:, off:ch], ones128[:],
                                     et[:, off:ch], start=st, stop=sp)
                    nc.tensor.matmul(y_ps[:, off:ch],
                                     vv[:, t * HD:(t + 1) * HD],
                                     et[:, off:ch], start=st, stop=sp)
                # Copy d out of PSUM promptly (frees the bank for the next
                # head) and take the fast approximate reciprocal in SBUF.
                d_sb = npool.tile([128, ch], F32, tag="dsb", name="dsb")
                nc.vector.tensor_copy(d_sb[:], ps_d[:])
                rec = npool.tile([128, ch], F32, tag="rec", name="rec")
                nc.vector.reciprocal_approx_fast(rec[:], d_sb[:])
                nc.vector.tensor_mul(
                    yT[h][:, j * ch:(j + 1) * ch], y_ps[:], rec[:])

        with tc.tile_pool(name="qkv_ps", bufs=1, space="PSUM") as qkv_ps:
            for j in range(nJ):
                qTc = [qtpool.tile([HD, ch], BF16, tag=f"qt{h}",
                                   name=f"qt{h}") for h in range(hq)]
                qkv_chunk(j, qkv_ps, qTc)
                attn_chunk(j, qTc)

        # ---- output projection (row-parallel wo partial sums) ----
        opool = ctx.enter_context(tc.tile_pool(name="opool", bufs=3))
        with tc.tile_pool(name="wo_ps", bufs=1, space="PSUM") as wo_ps:
            for j in range(nJ):
                for g in range(nD // 4):
                    last = (j == nJ - 1 and g == nD // 4 - 1)
                    og = opool.tile([128, 4, ch], BF16, tag="og", name="og")
                    for i in range(4):
                        dt = g * 4 + i
                        ps_o = wo_ps.tile([128, ch], F32, tag="pso",
                                          name="pso", bufs=3)
                        for o in range(hq):
                            nc.tensor.matmul(
                                ps_o[:], wosb[:, o, dt * 128:(dt + 1) * 128],
                                yT[o][:, j * ch:(j + 1) * ch],
                                start=(o == 0), stop=(o == hq - 1))
                        if last:
                            # final group: split each evacuation across both
                            # engines to shorten the kernel tail
                            h2c = ch // 2
                            nc.vector.tensor_copy(og[:, i, 0:h2c],
                                                  ps_o[:, 0:h2c])
                            nc.scalar.copy(og[:, i, h2c:ch], ps_o[:, h2c:ch])
                        elif dt % 2:
                            nc.scalar.copy(og[:, i, :], ps_o[:])
                        else:
                            nc.vector.tensor_copy(og[:, i, :], ps_o[:])
                    if last:
                        nc.scalar.dma_start(
                            outT_r[:, g * 4:g * 4 + 2, j * ch:(j + 1) * ch],
                            og[:, 0:2, :])
                        nc.scalar.dma_start(
                            outT_r[:, g * 4 + 2:g * 4 + 4,
                                   j * ch:(j + 1) * ch],
                            og[:, 2:4, :])
                    else:
                        nc.scalar.dma_start(
                            outT_r[:, g * 4:(g + 1) * 4,
                                   j * ch:(j + 1) * ch],
                            og[:])
    nc.compile()
    return nc


def _deinterleave_perm(hd):
    """Row permutation putting even indices first, odd second."""
    return np.concatenate([np.arange(0, hd, 2), np.arange(1, hd, 2)])


def host_prep(x, wq, wk, wv, wo, freqs_cos, freqs_sin,
              n_cores=N_CORES, hq=HQ, n_kv=N_KV_HEADS):
    """Build the per-core input maps (numpy, host-side)."""
    import ml_dtypes

    BF = ml_dtypes.bfloat16
    HD = HEAD_DIM
    D = x.shape[-1]
    S = x.shape[-2]
    M = hq * HD
    R = CH // 128
    x = np.asarray(x, np.float32).reshape(S, D)
    wq = np.asarray(wq, np.float32)
    wk = np.asarray(wk, np.float32)
    wv = np.asarray(wv, np.float32)
    wo = np.asarray(wo, np.float32)
    fc = np.asarray(freqs_cos, np.float32)
    fs = np.asarray(freqs_sin, np.float32)

    perm = _deinterleave_perm(HD)
    wq = wq * np.float32(SCALE)   # fold softmax scale into q projection
    # xP[j*4+qt, p, dd*CH+f] = x[j*CH+f, (qt*8+dd)*128+p] — one contiguous
    # 8KB run per (chunk-quarter, partition) so the DMA hits full HBM BW.
    nJ = S // CH
    xP = np.ascontiguousarray(
        x.T.reshape(4, 8, 128, nJ, CH).transpose(3, 0, 2, 1, 4)
        .reshape(nJ * 4, 128, 8 * CH)).astype(BF)
    cosP = np.ascontiguousarray(np.concatenate([fc.T, fc.T], 0)).astype(BF)
    sinP = np.ascontiguousarray(np.concatenate([-fs.T, fs.T], 0)).astype(BF)
    # mask[t, r*CH + s] = 1 if 128*r + t <= s else 0
    tt = np.arange(128)[:, None]
    ss = np.arange(CH)[None, :]
    maskD = np.concatenate(
        [(128 * r + tt <= ss).astype(np.float32) for r in range(R)], axis=1)
    maskD = np.ascontiguousarray(maskD).astype(BF)            # [128, R*CH]
    constD = np.concatenate(
        [np.eye(128, dtype=np.float32), np.ones((128, 128), np.float32)],
        axis=1).astype(BF)                                    # [128, 256]

    nD = D // 128
    in_maps = []
    for c in range(n_cores):
        wq_c = wq[c * M:(c + 1) * M, :].reshape(hq, HD, D)[:, perm, :]
        wq_c = wq_c.reshape(M, D)
        wk_c = wk[c * HD:(c + 1) * HD, :][perm, :]
        wv_c = wv[c * HD:(c + 1) * HD, :]
        wqkvT = np.concatenate([wq_c, wk_c, wv_c], axis=0).T  # [D, 768]
        # wP[o, p, d*128+f] = wqkvT[d*128+p, o*128+f]
        wP = np.ascontiguousarray(
            wqkvT.reshape(nD, 128, 6, 128).transpose(2, 1, 0, 3)
            .reshape(6, 128, nD * 128)).astype(BF)
        woT = np.ascontiguousarray(wo[:, c * M:(c + 1) * M].T).astype(BF)
        in_maps.append({
            "xP": xP, "wP": wP, "woT": woT, "constD": constD,
            "cosP": cosP, "sinP": sinP, "maskD": maskD,
        })
    return in_maps


_NC_CACHE = {}


def _get_module():
    if "nc" not in _NC_CACHE:
        _NC_CACHE["nc"] = build_module()
    return _NC_CACHE["nc"]


def run_on_cores(in_maps, trace=False):
    nc = _get_module()
    res = bass_utils.run_bass_kernel_spmd(
        nc, in_maps, core_ids=list(range(len(in_maps))), trace=trace)
    return res


def kernel(x, wq, wk, wv, wo, freqs_cos, freqs_sin):
    in_maps = host_prep(x, wq, wk, wv, wo, freqs_cos, freqs_sin)
    res = run_on_cores(in_maps)
    acc = None
    for r in res.results:
        o = np.asarray(r["outT"], dtype=np.float64)
        acc = o if acc is None else acc + o
    out = acc.T.astype(np.float32).reshape(1, SEQ, DIM)
    return out
